# revision 17
# baseline (speedup 1.0000x reference)
"""Trainium2 Bass kernel for nn_BiaffineChart.

Computes, for x_l, x_r [1, 4096, 1024], mask [4096, 4096] (bool),
matrix [1024, 1024], wl/wr [1, 1024], bias/bl/br [1]:

    xm     = x_l @ matrix                       # [1, n, d]
    x      = xm @ x_r^T + bias                  # [1, n, n]
    x     += lin_l(x_l) + lin_r(x_r)^T          # row + col vectors
    x      = relu(x)[0]                         # [n, n]
    scores = where(mask, x, 0)
    return (scores, x)

Sharding: rows of x_l / mask / outputs split across 8 NeuronCores
(sequence parallel); matrix / wl / wr / x_r replicated.

Per-core design (512 rows):
  All matmul operands live in HBM as fp16, pre-transposed on the host,
  so the PE does zero transposes and HBM traffic is halved vs fp32:
    loads  xlT [1024, 512] + mat [1024, 1024] + xrT [1024, 4096] = 11 MB
    stores x / scores as fp16                                    =  8 MB
  mm1 (xmT[r,m] = sum_l mat[l,r] xlT[l,m]) accumulates lt-outer across
  all 8 PSUM banks so it starts as soon as the first 128-row chunk of
  xlT/mat lands.  Evictions add wr[r] per partition, so mm2 picks up
  the lin_r column term for free:
      (xm[m,:] + wr) . x_r[n,:] = xm.x_r + lin_r[n]
  mm2 streams 1024-column blocks of xrT (2 KB DMA lines); the ScalarE
  relu eviction adds lin_l[m]+bias per partition; VectorE applies the
  mask (cast u8->fp16 by the SWDGE DMA on load).
  DMA queues: ALL loads + scores stores on qSP (sync) -- a single
  ordered queue so mm1's chunk stream is never preempted (splitting
  loads across queues measurably stalls mm1 and re-throttles the PE
  clock); x stores on qAct (scalar); mask on the gpsimd SWDGE queue.
  The last column block stores per 128-row slice on alternating queues
  to shrink the drain tail.

Measured end-to-end relative error vs the fp32 reference: ~4e-4.
"""

import os
import sys

import numpy as np

for _p in ("/opt/trn_rl_repo", "/opt/pypackages"):
    if _p not in sys.path:
        sys.path.append(_p)

from contextlib import ExitStack

import concourse.bass as bass
import concourse.tile as tile
from concourse import bacc
from concourse import mybir
from concourse.bass_utils import run_bass_kernel_spmd

N = 4096          # sequence length (rows and cols of the chart)
D = 1024          # feature dim
NCORES = 8
MSH = N // NCORES # rows per core = 512
P = 128           # partitions
KT = D // P       # 8 k-tiles of 128
MT = MSH // P     # 4 m-tiles per core
NBLK = 8          # 512-wide column blocks (mask / store granularity)
NF = N // NBLK    # 512
XB = 4            # 1024-wide xrT load blocks
XF = N // XB      # 1024

F32 = mybir.dt.float32
F16 = mybir.dt.float16
U8 = mybir.dt.uint8


def build_bass():
    nc = bacc.Bacc(name="biaffine_chart")

    xlT_d = nc.dram_tensor("xlT", [D, MSH], F16, kind="ExternalInput")
    xrT_d = nc.dram_tensor("xrT", [D, N], F16, kind="ExternalInput")
    mk_d = nc.dram_tensor("mk", [MSH, N], U8, kind="ExternalInput")
    mat_d = nc.dram_tensor("mat", [D, D], F16, kind="ExternalInput")
    # wb = [wrT | bias_col]: wr as columns, then lin_l+bias per partition
    wb_d = nc.dram_tensor("wb", [P, KT + MT], F32, kind="ExternalInput")

    sc_d = nc.dram_tensor("scores", [MSH, N], F16, kind="ExternalOutput")
    x_d = nc.dram_tensor("xout", [MSH, N], F16, kind="ExternalOutput")

    # partitioned views: leading index = tile*128 + partition
    xlT_v = xlT_d.rearrange("(lt p) m -> p lt m", p=P)   # [128, 8, 512]
    xrT_v = xrT_d.rearrange("(rt p) n -> p rt n", p=P)   # [128, 8, 4096]
    mat_v = mat_d.rearrange("(lt p) r -> p lt r", p=P)   # [128, 8, 1024]
    mk_v = mk_d.rearrange("(mo p) n -> p mo n", p=P)     # [128, 4, 4096]
    sc_v = sc_d.rearrange("(mo p) n -> p mo n", p=P)
    x_v = x_d.rearrange("(mo p) n -> p mo n", p=P)

    with tile.TileContext(nc) as tc, ExitStack() as ctx:
        consts = ctx.enter_context(tc.tile_pool(name="consts", bufs=1))
        xmT_pool = ctx.enter_context(tc.tile_pool(name="xmTp", bufs=1))
        xrT_pool = ctx.enter_context(tc.tile_pool(name="xrTp", bufs=3))
        mk_pool = ctx.enter_context(tc.tile_pool(name="mkp", bufs=2))
        xo_pool = ctx.enter_context(tc.tile_pool(name="xop", bufs=2))
        so_pool = ctx.enter_context(tc.tile_pool(name="sop", bufs=2))
        ps_pool = ctx.enter_context(tc.tile_pool(name="psp", bufs=8, space="PSUM"))

        # PE warm-up: the HAM clock gate starts throttled (1.2 GHz) and
        # needs ~3.4us of sustained matmul activity to release.  Burn the
        # initial DMA wait on scratch f32 matmuls so mm1 runs at 2.4 GHz.
        warm_w = consts.tile([P, P], F32)
        nc.vector.memset(warm_w[:], 1.0)
        warm_sb = consts.tile([P, NF], F32)
        nc.vector.memset(warm_sb[:], 1.0)
        warm_ps = ps_pool.tile([P, NF], F32, tag="mm")
        for _ in range(4):
            nc.tensor.matmul(warm_ps[:], warm_w[:], warm_sb[:], start=True, stop=True)

        # xlT / mat streamed in progressively larger lt-chunk groups on
        # two HW queues (xlT+wb on qSP, mat on qAct), sized so each group
        # lands just ahead of mm1's lt-outer consumption.  The DMA *issue*
        # instructions cost ~0.6us each on the issuing engine, so few
        # large-group issues beat 16 per-chunk issues (which starve mm1).
        # The early phase is chip-HBM-bound (all 8 cores pull their mm1
        # working sets simultaneously) and each queue only gets a ~150-200
        # GB/s share, so mat is spread across all three queues with the
        # latest-needed chunks on the slowest (SWDGE) queue.
        xlT_sb = consts.tile([P, KT, MSH], F16)
        mat_sb = consts.tile([P, KT, D], F16)
        nc.sync.dma_start(xlT_sb[:, 0, :], xlT_v[:, 0, :])
        nc.scalar.dma_start(mat_sb[:, 0, :], mat_v[:, 0, :])
        nc.gpsimd.dma_start(mat_sb[:, 5:KT, :], mat_v[:, 5:KT, :])
        nc.sync.dma_start(xlT_sb[:, 1:4, :], xlT_v[:, 1:4, :])
        nc.scalar.dma_start(mat_sb[:, 1:3, :], mat_v[:, 1:3, :])
        nc.sync.dma_start(mat_sb[:, 3:5, :], mat_v[:, 3:5, :])
        nc.sync.dma_start(xlT_sb[:, 4:KT, :], xlT_v[:, 4:KT, :])
        wb_sb = consts.tile([P, KT + MT], F32)
        nc.sync.dma_start(wb_sb[:], wb_d[:])

        # xrT blocks queued behind the mat chunks on qAct.  Block 0 lands
        # as two 512-col halves so mm2's first chains only wait on the
        # first half.
        xr_tiles = []
        xr0 = xrT_pool.tile([P, KT, XF], F16, tag="xrT", name="xr_blk0")
        nc.scalar.dma_start(xr0[:, :, 0:NF], xrT_v[:, :, 0:NF])
        nc.scalar.dma_start(xr0[:, :, NF:XF], xrT_v[:, :, NF:XF])
        xr_tiles.append(xr0)
        xr1 = xrT_pool.tile([P, KT, XF], F16, tag="xrT", name="xr_blk1")
        nc.scalar.dma_start(xr1[:], xrT_v[:, :, XF:2 * XF])
        xr_tiles.append(xr1)

        # mm1, lt-outer: all 8 rt chains accumulate in parallel across the
        # 8 PSUM banks; chain rt consumes only chunk lt each step.  On the
        # final lt the chains close rt-ascending with the wr-add eviction
        # (f32 PSUM -> fp16 xmT) pipelined right behind each stop.
        xmT = xmT_pool.tile([P, KT, MSH], F16)
        ps_mm1 = [
            ps_pool.tile([P, MSH], F32, tag="mm", name=f"ps_mm1_{rt}")
            for rt in range(KT)
        ]
        for lt in range(KT):
            last = lt == KT - 1
            for rt in range(KT):
                nc.tensor.matmul(
                    ps_mm1[rt][:],
                    mat_sb[:, lt, rt * P:(rt + 1) * P],
                    xlT_sb[:, lt, :],
                    start=(lt == 0),
                    stop=last,
                )
                if last:
                    nc.vector.tensor_scalar_add(
                        xmT[:, rt, :], ps_mm1[rt][:], wb_sb[:, rt:rt + 1]
                    )

        # ---- main loop: 4 xrT load blocks x 2 column sub-blocks ----
        for xb in range(XB):
            if xb + 2 < XB:
                xr_after = xrT_pool.tile(
                    [P, KT, XF], F16, tag="xrT", name=f"xr_blk{xb + 2}")
                nc.scalar.dma_start(
                    xr_after[:], xrT_v[:, :, (xb + 2) * XF:(xb + 3) * XF]
                )
                xr_tiles.append(xr_after)
            xr_cur = xr_tiles[xb]

            for nh in range(2):
                nb = 2 * xb + nh
                last_nb = nb == NBLK - 1
                # mask block, cast u8 -> fp16 by the SWDGE DMA
                mk_sb = mk_pool.tile([P, MT, NF], F16, tag="mk")
                nc.gpsimd.dma_start(mk_sb[:], mk_v[:, :, nb * NF:(nb + 1) * NF])

                x_st = xo_pool.tile([P, MT, NF], F16, tag="xo")
                s_st = so_pool.tile([P, MT, NF], F16, tag="so")
                for mt in range(MT):
                    ps = ps_pool.tile([P, NF], F32, tag="mm")
                    for rt in range(KT):
                        nc.tensor.matmul(
                            ps[:],
                            xmT[:, rt, mt * P:(mt + 1) * P],
                            xr_cur[:, rt, nh * NF:(nh + 1) * NF],
                            start=(rt == 0),
                            stop=(rt == KT - 1),
                        )
                    nc.scalar.activation(
                        x_st[:, mt, :], ps[:],
                        mybir.ActivationFunctionType.Relu,
                        bias=wb_sb[:, KT + mt:KT + mt + 1],
                    )
                    if last_nb:
                        # tail: mult + store per 128-row slice so earlier
                        # slices' transfers overlap the remaining chains.
                        # The final slice swaps store queues (x on qSP,
                        # scores on qAct) so its relu isn't stuck behind
                        # x-store issue instructions on the scalar engine
                        # and the two last transfers land on empty queues.
                        nc.vector.tensor_mul(
                            s_st[:, mt, :], x_st[:, mt, :], mk_sb[:, mt, :])
                        cs = nb * NF
                        if mt < MT - 1:
                            nc.scalar.dma_start(
                                x_v[:, mt, cs:cs + NF], x_st[:, mt, :])
                            nc.sync.dma_start(
                                sc_v[:, mt, cs:cs + NF], s_st[:, mt, :])
                        else:
                            nc.sync.dma_start(
                                x_v[:, mt, cs:cs + NF], x_st[:, mt, :])
                            nc.scalar.dma_start(
                                sc_v[:, mt, cs:cs + NF], s_st[:, mt, :])
                if not last_nb:
                    nc.vector.tensor_mul(s_st[:], x_st[:], mk_sb[:])
                    # x on qAct (scalar), scores on qSP (sync)
                    nc.scalar.dma_start(
                        x_v[:, :, nb * NF:(nb + 1) * NF], x_st[:])
                    nc.sync.dma_start(
                        sc_v[:, :, nb * NF:(nb + 1) * NF], s_st[:])

    nc.compile()
    return nc


_NC_CACHE = None

# test-harness knobs (the grading harness just calls kernel())
TRACE = False
TRACE_KW = {}
LAST_RESULTS = None


def _get_nc():
    global _NC_CACHE
    if _NC_CACHE is None:
        _NC_CACHE = build_bass()
    return _NC_CACHE


def kernel(x_l, x_r, mask, matrix, bias, wl, bl, wr, br, s_ind=0, **_):
    x_l2 = np.asarray(x_l, dtype=np.float32).reshape(N, D)
    x_r2 = np.asarray(x_r, dtype=np.float32).reshape(N, D)
    mat32 = np.asarray(matrix, dtype=np.float32)

    xlT = np.ascontiguousarray(x_l2.T.astype(np.float16))    # [D, N]
    xrT = np.ascontiguousarray(x_r2.T.astype(np.float16))    # [D, N]
    mat16 = np.ascontiguousarray(mat32.astype(np.float16))   # [D, D]
    mask_u8 = np.ascontiguousarray(np.asarray(mask)).astype(np.uint8)

    wr_v = np.asarray(wr, dtype=np.float32).reshape(D)
    wrT = wr_v.reshape(KT, P).T                              # [P, KT]

    c0 = float(np.asarray(bias).ravel()[0]) \
        + float(np.asarray(bl).ravel()[0]) \
        + float(np.asarray(br).ravel()[0])
    lin_l = x_l2 @ np.asarray(wl, dtype=np.float32).reshape(D) + c0  # [N]

    nc = _get_nc()
    in_maps = []
    for c in range(NCORES):
        sl = slice(c * MSH, (c + 1) * MSH)
        bc = lin_l[sl].reshape(MT, P).T                      # [P, MT]
        wb = np.ascontiguousarray(
            np.concatenate([wrT, bc], axis=1).astype(np.float32))
        in_maps.append({
            "xlT": np.ascontiguousarray(xlT[:, sl]),
            "xrT": xrT,
            "mk": mask_u8[sl],
            "mat": mat16,
            "wb": wb,
        })

    res = run_bass_kernel_spmd(
        nc, in_maps, core_ids=list(range(NCORES)), trace=TRACE, **TRACE_KW
    )
    global LAST_RESULTS
    LAST_RESULTS = res
    scores = np.concatenate(
        [r["scores"].astype(np.float32) for r in res.results], axis=0)
    x = np.concatenate(
        [r["xout"].astype(np.float32) for r in res.results], axis=0)
    return (scores, x)


# revision 18
# speedup vs baseline: 1.0564x; 1.0564x over previous
"""Trainium2 Bass kernel for nn_BiaffineChart.

Computes, for x_l, x_r [1, 4096, 1024], mask [4096, 4096] (bool),
matrix [1024, 1024], wl/wr [1, 1024], bias/bl/br [1]:

    xm     = x_l @ matrix                       # [1, n, d]
    x      = xm @ x_r^T + bias                  # [1, n, n]
    x     += lin_l(x_l) + lin_r(x_r)^T          # row + col vectors
    x      = relu(x)[0]                         # [n, n]
    scores = where(mask, x, 0)
    return (scores, x)

Sharding: rows of x_l / mask / outputs split across 8 NeuronCores
(sequence parallel); matrix / wl / wr / x_r replicated.

Per-core design (512 rows):
  All matmul operands live in HBM as fp16, pre-transposed on the host,
  so the PE does zero transposes and HBM traffic is halved vs fp32:
    loads  xlT [1024, 512] + mat [1024, 1024] + xrT [1024, 4096] = 11 MB
    stores x / scores as fp16                                    =  8 MB
  mm1 (xmT[r,m] = sum_l mat[l,r] xlT[l,m]) accumulates lt-outer across
  all 8 PSUM banks so it starts as soon as the first 128-row chunk of
  xlT/mat lands.  Evictions add wr[r] per partition, so mm2 picks up
  the lin_r column term for free:
      (xm[m,:] + wr) . x_r[n,:] = xm.x_r + lin_r[n]
  mm2 streams 1024-column blocks of xrT (2 KB DMA lines); the ScalarE
  relu eviction adds lin_l[m]+bias per partition; VectorE applies the
  mask (cast u8->fp16 by the SWDGE DMA on load).
  DMA queues: ALL loads + scores stores on qSP (sync) -- a single
  ordered queue so mm1's chunk stream is never preempted (splitting
  loads across queues measurably stalls mm1 and re-throttles the PE
  clock); x stores on qAct (scalar); mask on the gpsimd SWDGE queue.
  The last column block stores per 128-row slice on alternating queues
  to shrink the drain tail.

Measured end-to-end relative error vs the fp32 reference: ~4e-4.
"""

import os
import sys

import numpy as np

for _p in ("/opt/trn_rl_repo", "/opt/pypackages"):
    if _p not in sys.path:
        sys.path.append(_p)

from contextlib import ExitStack

import concourse.bass as bass
import concourse.tile as tile
from concourse import bacc
from concourse import mybir
from concourse.bass_utils import run_bass_kernel_spmd

N = 4096          # sequence length (rows and cols of the chart)
D = 1024          # feature dim
NCORES = 8
MSH = N // NCORES # rows per core = 512
P = 128           # partitions
KT = D // P       # 8 k-tiles of 128
MT = MSH // P     # 4 m-tiles per core
NBLK = 8          # 512-wide column blocks (mask / store granularity)
NF = N // NBLK    # 512
XB = 4            # 1024-wide xrT load blocks
XF = N // XB      # 1024

F32 = mybir.dt.float32
F16 = mybir.dt.float16
U8 = mybir.dt.uint8


def build_bass():
    nc = bacc.Bacc(name="biaffine_chart")

    xlT_d = nc.dram_tensor("xlT", [D, MSH], F16, kind="ExternalInput")
    xrT_d = nc.dram_tensor("xrT", [D, N], F16, kind="ExternalInput")
    mk_d = nc.dram_tensor("mk", [MSH, N], U8, kind="ExternalInput")
    mat_d = nc.dram_tensor("mat", [D, D], F16, kind="ExternalInput")
    # wb = [wrT | bias_col]: wr as columns, then lin_l+bias per partition
    wb_d = nc.dram_tensor("wb", [P, KT + MT], F32, kind="ExternalInput")

    sc_d = nc.dram_tensor("scores", [MSH, N], F16, kind="ExternalOutput")
    x_d = nc.dram_tensor("xout", [MSH, N], F16, kind="ExternalOutput")

    # partitioned views: leading index = tile*128 + partition
    xlT_v = xlT_d.rearrange("(lt p) m -> p lt m", p=P)   # [128, 8, 512]
    xrT_v = xrT_d.rearrange("(rt p) n -> p rt n", p=P)   # [128, 8, 4096]
    mat_v = mat_d.rearrange("(lt p) r -> p lt r", p=P)   # [128, 8, 1024]
    mk_v = mk_d.rearrange("(mo p) n -> p mo n", p=P)     # [128, 4, 4096]
    sc_v = sc_d.rearrange("(mo p) n -> p mo n", p=P)
    x_v = x_d.rearrange("(mo p) n -> p mo n", p=P)

    with tile.TileContext(nc) as tc, ExitStack() as ctx:
        consts = ctx.enter_context(tc.tile_pool(name="consts", bufs=1))
        xmT_pool = ctx.enter_context(tc.tile_pool(name="xmTp", bufs=1))
        xrT_pool = ctx.enter_context(tc.tile_pool(name="xrTp", bufs=3))
        mk_pool = ctx.enter_context(tc.tile_pool(name="mkp", bufs=2))
        xo_pool = ctx.enter_context(tc.tile_pool(name="xop", bufs=2))
        so_pool = ctx.enter_context(tc.tile_pool(name="sop", bufs=2))
        ps_pool = ctx.enter_context(tc.tile_pool(name="psp", bufs=8, space="PSUM"))

        # PE warm-up: the HAM clock gate starts throttled (1.2 GHz) and
        # needs ~3.4us of sustained matmul activity to release.  Burn the
        # initial DMA wait on scratch f32 matmuls so mm1 runs at 2.4 GHz.
        warm_w = consts.tile([P, P], F32)
        nc.vector.memset(warm_w[:], 1.0)
        warm_sb = consts.tile([P, NF], F32)
        nc.vector.memset(warm_sb[:], 1.0)
        warm_ps = ps_pool.tile([P, NF], F32, tag="mm")
        for _ in range(4):
            nc.tensor.matmul(warm_ps[:], warm_w[:], warm_sb[:], start=True, stop=True)

        # xlT / mat streamed in progressively larger lt-chunk groups on
        # two HW queues (xlT+wb on qSP, mat on qAct), sized so each group
        # lands just ahead of mm1's lt-outer consumption.  The DMA *issue*
        # instructions cost ~0.6us each on the issuing engine, so few
        # large-group issues beat 16 per-chunk issues (which starve mm1).
        # The early phase is chip-HBM-bound (all 8 cores pull their mm1
        # working sets simultaneously), so the chunk streams ride all
        # three queues in need-order: xlT on qSP, even mat chunks on
        # qAct, odd mat chunks on the SWDGE queue.
        xlT_sb = consts.tile([P, KT, MSH], F16)
        mat_sb = consts.tile([P, KT, D], F16)
        nc.sync.dma_start(xlT_sb[:, 0, :], xlT_v[:, 0, :])
        nc.scalar.dma_start(mat_sb[:, 0, :], mat_v[:, 0, :])
        wb_sb = consts.tile([P, KT + MT], F32)
        nc.sync.dma_start(wb_sb[:], wb_d[:])
        for lt in range(1, KT):
            nc.sync.dma_start(xlT_sb[:, lt, :], xlT_v[:, lt, :])
            if lt % 2:
                nc.gpsimd.dma_start(mat_sb[:, lt, :], mat_v[:, lt, :])
            else:
                nc.scalar.dma_start(mat_sb[:, lt, :], mat_v[:, lt, :])

        # xrT blocks queued behind the mat chunks on qAct.  Block 0 lands
        # as two 512-col halves so mm2's first chains only wait on the
        # first half.
        xr_tiles = []
        xr0 = xrT_pool.tile([P, KT, XF], F16, tag="xrT", name="xr_blk0")
        nc.scalar.dma_start(xr0[:, :, 0:NF], xrT_v[:, :, 0:NF])
        nc.scalar.dma_start(xr0[:, :, NF:XF], xrT_v[:, :, NF:XF])
        xr_tiles.append(xr0)
        xr1 = xrT_pool.tile([P, KT, XF], F16, tag="xrT", name="xr_blk1")
        nc.scalar.dma_start(xr1[:], xrT_v[:, :, XF:2 * XF])
        xr_tiles.append(xr1)

        # mm1, lt-outer: all 8 rt chains accumulate in parallel across the
        # 8 PSUM banks; chain rt consumes only chunk lt each step.  On the
        # final lt the chains close rt-ascending with the wr-add eviction
        # (f32 PSUM -> fp16 xmT) pipelined right behind each stop.
        xmT = xmT_pool.tile([P, KT, MSH], F16)
        ps_mm1 = [
            ps_pool.tile([P, MSH], F32, tag="mm", name=f"ps_mm1_{rt}")
            for rt in range(KT)
        ]
        for lt in range(KT):
            last = lt == KT - 1
            for rt in range(KT):
                nc.tensor.matmul(
                    ps_mm1[rt][:],
                    mat_sb[:, lt, rt * P:(rt + 1) * P],
                    xlT_sb[:, lt, :],
                    start=(lt == 0),
                    stop=last,
                )
                if last:
                    nc.vector.tensor_scalar_add(
                        xmT[:, rt, :], ps_mm1[rt][:], wb_sb[:, rt:rt + 1]
                    )

        # ---- main loop: 4 xrT load blocks x 2 column sub-blocks ----
        for xb in range(XB):
            if xb + 2 < XB:
                xr_after = xrT_pool.tile(
                    [P, KT, XF], F16, tag="xrT", name=f"xr_blk{xb + 2}")
                nc.scalar.dma_start(
                    xr_after[:], xrT_v[:, :, (xb + 2) * XF:(xb + 3) * XF]
                )
                xr_tiles.append(xr_after)
            xr_cur = xr_tiles[xb]

            for nh in range(2):
                nb = 2 * xb + nh
                last_nb = nb == NBLK - 1
                # mask block, cast u8 -> fp16 by the SWDGE DMA
                mk_sb = mk_pool.tile([P, MT, NF], F16, tag="mk")
                nc.gpsimd.dma_start(mk_sb[:], mk_v[:, :, nb * NF:(nb + 1) * NF])

                x_st = xo_pool.tile([P, MT, NF], F16, tag="xo")
                s_st = so_pool.tile([P, MT, NF], F16, tag="so")
                for mt in range(MT):
                    ps = ps_pool.tile([P, NF], F32, tag="mm")
                    for rt in range(KT):
                        nc.tensor.matmul(
                            ps[:],
                            xmT[:, rt, mt * P:(mt + 1) * P],
                            xr_cur[:, rt, nh * NF:(nh + 1) * NF],
                            start=(rt == 0),
                            stop=(rt == KT - 1),
                        )
                    nc.scalar.activation(
                        x_st[:, mt, :], ps[:],
                        mybir.ActivationFunctionType.Relu,
                        bias=wb_sb[:, KT + mt:KT + mt + 1],
                    )
                    if last_nb:
                        # tail: mult + store per 128-row slice so earlier
                        # slices' transfers overlap the remaining chains.
                        # The final slice swaps store queues (x on qSP,
                        # scores on qAct) so its relu isn't stuck behind
                        # x-store issue instructions on the scalar engine
                        # and the two last transfers land on empty queues.
                        nc.vector.tensor_mul(
                            s_st[:, mt, :], x_st[:, mt, :], mk_sb[:, mt, :])
                        cs = nb * NF
                        if mt < MT - 1:
                            nc.scalar.dma_start(
                                x_v[:, mt, cs:cs + NF], x_st[:, mt, :])
                            nc.sync.dma_start(
                                sc_v[:, mt, cs:cs + NF], s_st[:, mt, :])
                        else:
                            nc.sync.dma_start(
                                x_v[:, mt, cs:cs + NF], x_st[:, mt, :])
                            nc.scalar.dma_start(
                                sc_v[:, mt, cs:cs + NF], s_st[:, mt, :])
                if not last_nb:
                    nc.vector.tensor_mul(s_st[:], x_st[:], mk_sb[:])
                    # x on qAct (scalar), scores on qSP (sync)
                    nc.scalar.dma_start(
                        x_v[:, :, nb * NF:(nb + 1) * NF], x_st[:])
                    nc.sync.dma_start(
                        sc_v[:, :, nb * NF:(nb + 1) * NF], s_st[:])

    nc.compile()
    return nc


_NC_CACHE = None

# test-harness knobs (the grading harness just calls kernel())
TRACE = False
TRACE_KW = {}
LAST_RESULTS = None


def _get_nc():
    global _NC_CACHE
    if _NC_CACHE is None:
        _NC_CACHE = build_bass()
    return _NC_CACHE


def kernel(x_l, x_r, mask, matrix, bias, wl, bl, wr, br, s_ind=0, **_):
    x_l2 = np.asarray(x_l, dtype=np.float32).reshape(N, D)
    x_r2 = np.asarray(x_r, dtype=np.float32).reshape(N, D)
    mat32 = np.asarray(matrix, dtype=np.float32)

    xlT = np.ascontiguousarray(x_l2.T.astype(np.float16))    # [D, N]
    xrT = np.ascontiguousarray(x_r2.T.astype(np.float16))    # [D, N]
    mat16 = np.ascontiguousarray(mat32.astype(np.float16))   # [D, D]
    mask_u8 = np.ascontiguousarray(np.asarray(mask)).astype(np.uint8)

    wr_v = np.asarray(wr, dtype=np.float32).reshape(D)
    wrT = wr_v.reshape(KT, P).T                              # [P, KT]

    c0 = float(np.asarray(bias).ravel()[0]) \
        + float(np.asarray(bl).ravel()[0]) \
        + float(np.asarray(br).ravel()[0])
    lin_l = x_l2 @ np.asarray(wl, dtype=np.float32).reshape(D) + c0  # [N]

    nc = _get_nc()
    in_maps = []
    for c in range(NCORES):
        sl = slice(c * MSH, (c + 1) * MSH)
        bc = lin_l[sl].reshape(MT, P).T                      # [P, MT]
        wb = np.ascontiguousarray(
            np.concatenate([wrT, bc], axis=1).astype(np.float32))
        in_maps.append({
            "xlT": np.ascontiguousarray(xlT[:, sl]),
            "xrT": xrT,
            "mk": mask_u8[sl],
            "mat": mat16,
            "wb": wb,
        })

    res = run_bass_kernel_spmd(
        nc, in_maps, core_ids=list(range(NCORES)), trace=TRACE, **TRACE_KW
    )
    global LAST_RESULTS
    LAST_RESULTS = res
    scores = np.concatenate(
        [r["scores"].astype(np.float32) for r in res.results], axis=0)
    x = np.concatenate(
        [r["xout"].astype(np.float32) for r in res.results], axis=0)
    return (scores, x)


# revision 19
# speedup vs baseline: 1.0764x; 1.0190x over previous
"""Trainium2 Bass kernel for nn_BiaffineChart.

Computes, for x_l, x_r [1, 4096, 1024], mask [4096, 4096] (bool),
matrix [1024, 1024], wl/wr [1, 1024], bias/bl/br [1]:

    xm     = x_l @ matrix                       # [1, n, d]
    x      = xm @ x_r^T + bias                  # [1, n, n]
    x     += lin_l(x_l) + lin_r(x_r)^T          # row + col vectors
    x      = relu(x)[0]                         # [n, n]
    scores = where(mask, x, 0)
    return (scores, x)

Sharding: rows of x_l / mask / outputs split across 8 NeuronCores
(sequence parallel); matrix / wl / wr / x_r replicated.

Per-core design (512 rows):
  All matmul operands live in HBM as fp16, pre-transposed on the host,
  so the PE does zero transposes and HBM traffic is halved vs fp32:
    loads  xlT [1024, 512] + mat [1024, 1024] + xrT [1024, 4096] = 11 MB
    stores x / scores as fp16                                    =  8 MB
  mm1 (xmT[r,m] = sum_l mat[l,r] xlT[l,m]) accumulates lt-outer across
  all 8 PSUM banks so it starts as soon as the first 128-row chunk of
  xlT/mat lands.  Evictions add wr[r] per partition, so mm2 picks up
  the lin_r column term for free:
      (xm[m,:] + wr) . x_r[n,:] = xm.x_r + lin_r[n]
  mm2 streams 1024-column blocks of xrT (2 KB DMA lines); the ScalarE
  relu eviction adds lin_l[m]+bias per partition; VectorE applies the
  mask (cast u8->fp16 by the SWDGE DMA on load).
  DMA queues (need-ordered; the early phase is chip-HBM-bound because
  all 8 cores pull their mm1 working sets at once): xlT chunks + wb +
  scores stores on qSP (sync); even mat chunks + xrT blocks + x stores
  on qAct (scalar); odd mat chunks + mask on the gpsimd SWDGE queue.
  The last column block stores per 128-row slice, the final slice on
  swapped queues, so the drain tail is two small parallel transfers.

Measured end-to-end relative error vs the fp32 reference: ~4e-4.
"""

import os
import sys

import numpy as np

for _p in ("/opt/trn_rl_repo", "/opt/pypackages"):
    if _p not in sys.path:
        sys.path.append(_p)

from contextlib import ExitStack

import concourse.bass as bass
import concourse.tile as tile
from concourse import bacc
from concourse import mybir
from concourse.bass_utils import run_bass_kernel_spmd

N = 4096          # sequence length (rows and cols of the chart)
D = 1024          # feature dim
NCORES = 8
MSH = N // NCORES # rows per core = 512
P = 128           # partitions
KT = D // P       # 8 k-tiles of 128
MT = MSH // P     # 4 m-tiles per core
NBLK = 8          # 512-wide column blocks (mask / store granularity)
NF = N // NBLK    # 512
XB = 4            # 1024-wide xrT load blocks
XF = N // XB      # 1024

F32 = mybir.dt.float32
F16 = mybir.dt.float16
U8 = mybir.dt.uint8


def build_bass():
    nc = bacc.Bacc(name="biaffine_chart")

    xlT_d = nc.dram_tensor("xlT", [D, MSH], F16, kind="ExternalInput")
    xrT_d = nc.dram_tensor("xrT", [D, N], F16, kind="ExternalInput")
    mk_d = nc.dram_tensor("mk", [MSH, N], U8, kind="ExternalInput")
    mat_d = nc.dram_tensor("mat", [D, D], F16, kind="ExternalInput")
    # wb = [wrT | bias_col]: wr as columns, then lin_l+bias per partition
    wb_d = nc.dram_tensor("wb", [P, KT + MT], F32, kind="ExternalInput")

    sc_d = nc.dram_tensor("scores", [MSH, N], F16, kind="ExternalOutput")
    x_d = nc.dram_tensor("xout", [MSH, N], F16, kind="ExternalOutput")

    # partitioned views: leading index = tile*128 + partition
    xlT_v = xlT_d.rearrange("(lt p) m -> p lt m", p=P)   # [128, 8, 512]
    xrT_v = xrT_d.rearrange("(rt p) n -> p rt n", p=P)   # [128, 8, 4096]
    mat_v = mat_d.rearrange("(lt p) r -> p lt r", p=P)   # [128, 8, 1024]
    mk_v = mk_d.rearrange("(mo p) n -> p mo n", p=P)     # [128, 4, 4096]
    sc_v = sc_d.rearrange("(mo p) n -> p mo n", p=P)
    x_v = x_d.rearrange("(mo p) n -> p mo n", p=P)

    with tile.TileContext(nc) as tc, ExitStack() as ctx:
        consts = ctx.enter_context(tc.tile_pool(name="consts", bufs=1))
        xmT_pool = ctx.enter_context(tc.tile_pool(name="xmTp", bufs=1))
        xrT_pool = ctx.enter_context(tc.tile_pool(name="xrTp", bufs=3))
        mk_pool = ctx.enter_context(tc.tile_pool(name="mkp", bufs=2))
        xo_pool = ctx.enter_context(tc.tile_pool(name="xop", bufs=2))
        so_pool = ctx.enter_context(tc.tile_pool(name="sop", bufs=2))
        ps_pool = ctx.enter_context(tc.tile_pool(name="psp", bufs=8, space="PSUM"))

        # PE warm-up: the HAM clock gate starts throttled (1.2 GHz) and
        # needs ~3.4us of sustained matmul activity to release.  Burn the
        # initial DMA wait on scratch f32 matmuls so mm1 runs at 2.4 GHz.
        warm_w = consts.tile([P, P], F32)
        nc.vector.memset(warm_w[:], 1.0)
        warm_sb = consts.tile([P, NF], F32)
        nc.vector.memset(warm_sb[:], 1.0)
        warm_ps = ps_pool.tile([P, NF], F32, tag="mm")
        for _ in range(4):
            nc.tensor.matmul(warm_ps[:], warm_w[:], warm_sb[:], start=True, stop=True)

        # xlT / mat streamed in progressively larger lt-chunk groups on
        # two HW queues (xlT+wb on qSP, mat on qAct), sized so each group
        # lands just ahead of mm1's lt-outer consumption.  The DMA *issue*
        # instructions cost ~0.6us each on the issuing engine, so few
        # large-group issues beat 16 per-chunk issues (which starve mm1).
        # The early phase is chip-HBM-bound (all 8 cores pull their mm1
        # working sets simultaneously), so the chunk streams ride all
        # three queues in need-order: xlT on qSP, even mat chunks on
        # qAct, odd mat chunks on the SWDGE queue.
        xlT_sb = consts.tile([P, KT, MSH], F16)
        mat_sb = consts.tile([P, KT, D], F16)
        nc.sync.dma_start(xlT_sb[:, 0, :], xlT_v[:, 0, :])
        nc.scalar.dma_start(mat_sb[:, 0, :], mat_v[:, 0, :])
        wb_sb = consts.tile([P, KT + MT], F32)
        nc.sync.dma_start(wb_sb[:], wb_d[:])
        for lt in range(1, KT):
            nc.sync.dma_start(xlT_sb[:, lt, :], xlT_v[:, lt, :])
            if lt % 2:
                nc.gpsimd.dma_start(mat_sb[:, lt, :], mat_v[:, lt, :])
            else:
                nc.scalar.dma_start(mat_sb[:, lt, :], mat_v[:, lt, :])

        # xrT blocks queued behind the mat chunks on qAct.  Block 0 lands
        # as two 512-col halves so mm2's first chains only wait on the
        # first half.
        xr_tiles = []
        xr0 = xrT_pool.tile([P, KT, XF], F16, tag="xrT", name="xr_blk0")
        nc.scalar.dma_start(xr0[:, :, 0:NF], xrT_v[:, :, 0:NF])
        nc.scalar.dma_start(xr0[:, :, NF:XF], xrT_v[:, :, NF:XF])
        xr_tiles.append(xr0)
        xr1 = xrT_pool.tile([P, KT, XF], F16, tag="xrT", name="xr_blk1")
        nc.scalar.dma_start(xr1[:], xrT_v[:, :, XF:2 * XF])
        xr_tiles.append(xr1)

        # mm1, lt-outer: all 8 rt chains accumulate in parallel across the
        # 8 PSUM banks; chain rt consumes only chunk lt each step.  On the
        # final lt the chains close rt-ascending with the wr-add eviction
        # (f32 PSUM -> fp16 xmT) pipelined right behind each stop.
        xmT = xmT_pool.tile([P, KT, MSH], F16)
        ps_mm1 = [
            ps_pool.tile([P, MSH], F32, tag="mm", name=f"ps_mm1_{rt}")
            for rt in range(KT)
        ]
        for lt in range(KT):
            last = lt == KT - 1
            for rt in range(KT):
                nc.tensor.matmul(
                    ps_mm1[rt][:],
                    mat_sb[:, lt, rt * P:(rt + 1) * P],
                    xlT_sb[:, lt, :],
                    start=(lt == 0),
                    stop=last,
                )
                if last:
                    nc.vector.tensor_scalar_add(
                        xmT[:, rt, :], ps_mm1[rt][:], wb_sb[:, rt:rt + 1]
                    )

        # ---- main loop: 4 xrT load blocks x 2 column sub-blocks ----
        for xb in range(XB):
            if xb + 2 < XB:
                xr_after = xrT_pool.tile(
                    [P, KT, XF], F16, tag="xrT", name=f"xr_blk{xb + 2}")
                nc.scalar.dma_start(
                    xr_after[:], xrT_v[:, :, (xb + 2) * XF:(xb + 3) * XF]
                )
                xr_tiles.append(xr_after)
            xr_cur = xr_tiles[xb]

            for nh in range(2):
                nb = 2 * xb + nh
                last_nb = nb == NBLK - 1
                # mask block, cast u8 -> fp16 by the SWDGE DMA
                mk_sb = mk_pool.tile([P, MT, NF], F16, tag="mk")
                nc.gpsimd.dma_start(mk_sb[:], mk_v[:, :, nb * NF:(nb + 1) * NF])

                x_st = xo_pool.tile([P, MT, NF], F16, tag="xo")
                s_st = so_pool.tile([P, MT, NF], F16, tag="so")
                for mt in range(MT):
                    ps = ps_pool.tile([P, NF], F32, tag="mm")
                    for rt in range(KT):
                        nc.tensor.matmul(
                            ps[:],
                            xmT[:, rt, mt * P:(mt + 1) * P],
                            xr_cur[:, rt, nh * NF:(nh + 1) * NF],
                            start=(rt == 0),
                            stop=(rt == KT - 1),
                        )
                    nc.scalar.activation(
                        x_st[:, mt, :], ps[:],
                        mybir.ActivationFunctionType.Relu,
                        bias=wb_sb[:, KT + mt:KT + mt + 1],
                    )
                    if last_nb:
                        # tail: mult + store per 128-row slice so earlier
                        # slices' transfers overlap the remaining chains.
                        # The final slice swaps store queues (x on qSP,
                        # scores on qAct) so its relu isn't stuck behind
                        # x-store issue instructions on the scalar engine
                        # and the two last transfers land on empty queues.
                        nc.vector.tensor_mul(
                            s_st[:, mt, :], x_st[:, mt, :], mk_sb[:, mt, :])
                        cs = nb * NF
                        if mt < MT - 1:
                            nc.scalar.dma_start(
                                x_v[:, mt, cs:cs + NF], x_st[:, mt, :])
                            nc.sync.dma_start(
                                sc_v[:, mt, cs:cs + NF], s_st[:, mt, :])
                        else:
                            nc.sync.dma_start(
                                x_v[:, mt, cs:cs + NF], x_st[:, mt, :])
                            nc.scalar.dma_start(
                                sc_v[:, mt, cs:cs + NF], s_st[:, mt, :])
                if not last_nb:
                    nc.vector.tensor_mul(s_st[:], x_st[:], mk_sb[:])
                    # x on qAct (scalar), scores on qSP (sync)
                    nc.scalar.dma_start(
                        x_v[:, :, nb * NF:(nb + 1) * NF], x_st[:])
                    nc.sync.dma_start(
                        sc_v[:, :, nb * NF:(nb + 1) * NF], s_st[:])

    nc.compile()
    return nc


_NC_CACHE = None

# test-harness knobs (the grading harness just calls kernel())
TRACE = False
TRACE_KW = {}
LAST_RESULTS = None


def _get_nc():
    global _NC_CACHE
    if _NC_CACHE is None:
        _NC_CACHE = build_bass()
    return _NC_CACHE


def kernel(x_l, x_r, mask, matrix, bias, wl, bl, wr, br, s_ind=0, **_):
    x_l2 = np.asarray(x_l, dtype=np.float32).reshape(N, D)
    x_r2 = np.asarray(x_r, dtype=np.float32).reshape(N, D)
    mat32 = np.asarray(matrix, dtype=np.float32)

    xlT = np.ascontiguousarray(x_l2.T.astype(np.float16))    # [D, N]
    xrT = np.ascontiguousarray(x_r2.T.astype(np.float16))    # [D, N]
    mat16 = np.ascontiguousarray(mat32.astype(np.float16))   # [D, D]
    mask_u8 = np.ascontiguousarray(np.asarray(mask)).astype(np.uint8)

    wr_v = np.asarray(wr, dtype=np.float32).reshape(D)
    wrT = wr_v.reshape(KT, P).T                              # [P, KT]

    c0 = float(np.asarray(bias).ravel()[0]) \
        + float(np.asarray(bl).ravel()[0]) \
        + float(np.asarray(br).ravel()[0])
    lin_l = x_l2 @ np.asarray(wl, dtype=np.float32).reshape(D) + c0  # [N]

    nc = _get_nc()
    in_maps = []
    for c in range(NCORES):
        sl = slice(c * MSH, (c + 1) * MSH)
        bc = lin_l[sl].reshape(MT, P).T                      # [P, MT]
        wb = np.ascontiguousarray(
            np.concatenate([wrT, bc], axis=1).astype(np.float32))
        in_maps.append({
            "xlT": np.ascontiguousarray(xlT[:, sl]),
            "xrT": xrT,
            "mk": mask_u8[sl],
            "mat": mat16,
            "wb": wb,
        })

    res = run_bass_kernel_spmd(
        nc, in_maps, core_ids=list(range(NCORES)), trace=TRACE, **TRACE_KW
    )
    global LAST_RESULTS
    LAST_RESULTS = res
    scores = np.concatenate(
        [r["scores"].astype(np.float32) for r in res.results], axis=0)
    x = np.concatenate(
        [r["xout"].astype(np.float32) for r in res.results], axis=0)
    return (scores, x)
